# revision 2
# baseline (speedup 1.0000x reference)
# Bass/Tile TRN2 kernel for nn_Conv1D_style: out = ((x * (cluster@style_L)) @ weight) * (cluster@style_R)
#
# Sharding: data-parallel over the batch dim. Each of the 8 cores gets a
# 1024-row slice of x/cluster and a full (replicated) weight/style_L/style_R.
#
# Per-core plan (M=1024 batch, K=4096 din, N=4096 dout), all matmuls bf16
# with fp32 PSUM accumulation:
#   aT[k] = xT[k] * (style_L[:, kslice].T @ clusterT)  -> bf16, SBUF-resident.
#   y[m,n] = sum_k aT[k][:, mslice].T @ W[k, nslice]   (32 accumulating MMs)
#   out[m,n] = y[m,n] * (clusterT[:, mslice].T @ style_R[:, nslice])
#
# The aT production is fused with the first n-block's accumulation: n=0,
# m=0..5 accumulate k-outer across 6 PSUM banks while aT streams in, so the
# prologue's PE time (~48us) covers the startup DMA burst (~14 MiB) below
# the per-core HBM roofline. The K=64 style matmuls (tmpLT/tmpR) are
# row-packed two at a time via tile_position into the upper/lower 64 PE
# rows. ~40 warmup matmuls on a memset tile run during the initial DMA
# latency so the PE HAM clock-gate is at 8/8 when real matmuls start.
#
# DMA layout: xT and W are pre-arranged on the host partition-major so each
# DMA lands 8-32 KiB contiguous per SBUF partition (big packets). The
# gating constants (clT, sL k-granule 0) and xT stream on the Activation
# HWDGE queue, W + outputs on the Sync queue, and the non-critical style
# granules (sL tail, sR) on the GpSimd SWDGE queue so the critical queues
# stay short at startup. Output is written bf16 (halves write traffic) and
# upcast to fp32 on the host.

import numpy as np
import ml_dtypes

B, DIN, DOUT, NCL = 8192, 4096, 4096, 64
NCORES = 8
MB = B // NCORES          # batch rows per core
P = 128
NT = 512                  # n tile (dout cols per matmul)
KT = DIN // P             # 32 k tiles
MT = MB // P              # 8 m tiles
NTS = DOUT // NT          # 8 n tiles
FUSED = 6                 # m tiles of n=0 accumulated during the aT prologue
XG = 4                    # k tiles per xT DMA granule
WARMUP = 40               # PE warmup matmuls (N=128) during startup DMA

_CACHE = {}
LAST = {}                 # exposes the most recent BassKernelResults for test harnesses


def _build_program():
    import concourse.bacc as bacc
    import concourse.mybir as mybir
    import concourse.tile as tile

    bf16 = mybir.dt.bfloat16
    f32 = mybir.dt.float32

    nc = bacc.Bacc(None, target_bir_lowering=False, debug=False)

    # xT: [granule, partition, k-in-granule, batch]; W: [n, partition, k, nt]
    # cluster/styles arrive duplicated: rows 64-127 = rows 0-63 (row packing).
    xT_d = nc.declare_dram_parameter("xT", [KT // XG, P, XG, MB], bf16, isOutput=False)
    clT_d = nc.declare_dram_parameter("clusterT", [P, MB], bf16, isOutput=False)
    w_d = nc.declare_dram_parameter("weight", [NTS, P, KT, NT], bf16, isOutput=False)
    sL_d = nc.declare_dram_parameter("style_L", [P, DIN], bf16, isOutput=False)
    sR_d = nc.declare_dram_parameter("style_R", [P, DOUT], bf16, isOutput=False)
    out_d = nc.declare_dram_parameter("out", [MB, DOUT], bf16, isOutput=True)

    H = NCL  # 64: row-pack halves

    with tile.TileContext(nc) as tc:
        with (
            tc.tile_pool(name="const", bufs=1) as const_pool,
            tc.tile_pool(name="atp", bufs=1) as at_pool,
            tc.tile_pool(name="wp", bufs=2) as w_pool,
            tc.tile_pool(name="xp", bufs=3) as x_pool,
            tc.tile_pool(name="evp", bufs=3) as ev_pool,
            # PSUM budget (8 banks): py 6 x [128,512] (y accumulators + body
            # tmpR psum) + pl 1 x [128,1024] (2 banks) = 8.
            tc.tile_pool(name="pyp", bufs=6, space="PSUM") as py_pool,
            tc.tile_pool(name="plp", bufs=1, space="PSUM") as pl_pool,
        ):
            # ---- constants. clT + the first sL granule gate the first
            # tmpLT matmul: they go first on the Activation queue, ahead of
            # the xT stream. sL granules 1-3 and sR are not needed for tens
            # of us -> GpSimd SWDGE queue keeps them off the critical path.
            clT = const_pool.tile([P, MB], bf16, name="clT")
            sL = const_pool.tile([P, DIN], bf16, name="sL")
            sR = const_pool.tile([P, DOUT], bf16, name="sR")
            warm = const_pool.tile([P, P], bf16, name="warm")
            nc.gpsimd.memset(warm[:], 0.5)
            nc.scalar.dma_start(clT[:], clT_d[:])
            SLG = DIN // 4
            nc.scalar.dma_start(sL[:, 0:SLG], sL_d[:, 0:SLG])
            for g in range(1, 4):
                nc.gpsimd.dma_start(
                    sL[:, g * SLG:(g + 1) * SLG], sL_d[:, g * SLG:(g + 1) * SLG]
                )
            nc.gpsimd.dma_start(sR[:, 0:DOUT // 2], sR_d[:, 0:DOUT // 2])
            nc.gpsimd.dma_start(sR[:, DOUT // 2:], sR_d[:, DOUT // 2:])

            # ---- PE warmup: keep the PE busy from program start until the
            # gating DMAs land, so HAM un-throttles to 8/8 before real work.
            # Results go to the pl psum slot and are never read (the first
            # real tmpLT matmul overwrites them, WAW-ordered on the PE).
            wps = pl_pool.tile([P, MB], f32, name="wps", tag="pl")
            for i in range(WARMUP):
                nc.tensor.matmul(
                    wps[:, 0:P], warm[:], warm[:], start=True, stop=True
                )

            def tmpr_pair(n, m, psum_src="py"):
                """Row-packed pair: tmpR tiles for (m, m+1) at n, staged to SBUF.

                psum_src="pl" borrows the pl-pool slot (two banks) instead of
                two py slots — required around the fused prologue where all
                six py slots are held by the open accumulators.
                """
                if psum_src == "pl":
                    prp = pl_pool.tile([P, MB], f32, name=f"prf{n}_{m}", tag="pl")
                    pra, prb = prp[:, 0:NT], prp[:, NT:MB]
                else:
                    pra = py_pool.tile([P, NT], f32, name=f"pr{n}_{m}", tag="py")
                    prb = py_pool.tile([P, NT], f32, name=f"pr{n}_{m + 1}", tag="py")
                nc.tensor.matmul(
                    pra[:],
                    clT[:H, m * P:(m + 1) * P],
                    sR[:H, n * NT:(n + 1) * NT],
                    start=True, stop=True, tile_position=(0, 0),
                )
                nc.tensor.matmul(
                    prb[:],
                    clT[H:, (m + 1) * P:(m + 2) * P],
                    sR[H:, n * NT:(n + 1) * NT],
                    start=True, stop=True, tile_position=(H, 0),
                )
                tra = ev_pool.tile([P, NT], f32, name=f"tr{n}_{m}", tag="tr", bufs=6)
                trb = ev_pool.tile([P, NT], f32, name=f"tr{n}_{m + 1}", tag="tr", bufs=6)
                nc.any.tensor_copy(out=tra[:], in_=pra[:])
                nc.any.tensor_copy(out=trb[:], in_=prb[:])
                return tra, trb

            def epilogue(n, m, py, tr, split=False):
                ot = ev_pool.tile([P, NT], bf16, name=f"ot{n}_{m}", tag="ot")
                if split:
                    hw = NT // 2
                    for h in range(2):
                        s = slice(h * hw, (h + 1) * hw)
                        nc.vector.tensor_mul(out=ot[:, s], in0=py[:, s], in1=tr[:, s])
                        nc.sync.dma_start(
                            out_d[m * P:(m + 1) * P,
                                  n * NT + h * hw:n * NT + (h + 1) * hw],
                            ot[:, s],
                        )
                else:
                    nc.vector.tensor_mul(out=ot[:], in0=py[:], in1=tr[:])
                    nc.sync.dma_start(
                        out_d[m * P:(m + 1) * P, n * NT:(n + 1) * NT], ot[:]
                    )

            # ---- W for n=0, in k-granules so the first fused MM is gated on
            # only 512 KiB ----
            w0 = w_pool.tile([P, KT, NT], bf16, name="w0", tag="wbig")
            for lo, hi in ((0, 4), (4, 16), (16, 32)):
                nc.sync.dma_start(w0[:, lo:hi, :], w_d[0, :, lo:hi, :])

            # ---- fused prologue: aT production + n0/m0..5 k-outer accumulation ----
            py_f = [
                py_pool.tile([P, NT], f32, name=f"py0_{m}", tag="py")
                for m in range(FUSED)
            ]
            at_tiles = []
            for g in range(KT // XG):
                xg = x_pool.tile([P, XG, MB], bf16, name=f"xg{g}", tag="xg")
                if g == 0:
                    # split so k=0 is gated on 256 KiB, not 1 MiB
                    nc.scalar.dma_start(xg[:, 0:1, :], xT_d[0][:, 0:1, :])
                    nc.scalar.dma_start(xg[:, 1:XG, :], xT_d[0][:, 1:XG, :])
                else:
                    nc.scalar.dma_start(xg[:], xT_d[g])
                for j in range(XG):
                    k = g * XG + j
                    # tmpLT: row-packed pair, both batch halves in one slot
                    pl = pl_pool.tile([P, MB], f32, name=f"pl{k}", tag="pl")
                    nc.tensor.matmul(
                        pl[:, 0:NT],
                        sL[:H, k * P:(k + 1) * P],
                        clT[:H, 0:NT],
                        start=True, stop=True, tile_position=(0, 0),
                    )
                    nc.tensor.matmul(
                        pl[:, NT:MB],
                        sL[H:, k * P:(k + 1) * P],
                        clT[H:, NT:MB],
                        start=True, stop=True, tile_position=(H, 0),
                    )
                    at_k = at_pool.tile([P, MB], bf16, name=f"at{k}", tag=f"at{k}")
                    nc.vector.tensor_mul(out=at_k[:], in0=xg[:, j, :], in1=pl[:])
                    at_tiles.append(at_k)
                    for m in range(FUSED):
                        nc.tensor.matmul(
                            py_f[m][:],
                            at_k[:, m * P:(m + 1) * P],
                            w0[:, k, :],
                            start=(k == 0), stop=(k == KT - 1),
                        )

            # ---- n=0 seam: tmpR pairs ride the pl slot; epilogues release
            # py banks as the m6/m7 body groups consume them ----
            tr_f = list(tmpr_pair(0, 0, psum_src="pl"))
            epilogue(0, 0, py_f[0], tr_f[0])
            epilogue(0, 1, py_f[1], tr_f[1])

            py6 = py_pool.tile([P, NT], f32, name="py0_6", tag="py")
            for k in range(KT):
                nc.tensor.matmul(
                    py6[:],
                    at_tiles[k][:, 6 * P:7 * P],
                    w0[:, k, :],
                    start=(k == 0), stop=(k == KT - 1),
                )
                if k == 7:
                    tr_f += tmpr_pair(0, 2, psum_src="pl")
                    epilogue(0, 2, py_f[2], tr_f[2])
                    epilogue(0, 3, py_f[3], tr_f[3])
                elif k == 15:
                    tr_f += tmpr_pair(0, 4, psum_src="pl")
                    epilogue(0, 4, py_f[4], tr_f[4])
                    epilogue(0, 5, py_f[5], tr_f[5])
                elif k == 23:
                    tr_f += tmpr_pair(0, 6, psum_src="pl")
            epilogue(0, 6, py6, tr_f[6])

            py7 = py_pool.tile([P, NT], f32, name="py0_7", tag="py")
            for k in range(KT):
                nc.tensor.matmul(
                    py7[:],
                    at_tiles[k][:, 7 * P:8 * P],
                    w0[:, k, :],
                    start=(k == 0), stop=(k == KT - 1),
                )
            epilogue(0, 7, py7, tr_f[7])

            # ---- standard m-pair body: two 32-MM groups with the packed tmpR
            # pair injected mid-group (the deep MM pipeline hides its
            # LDWEIGHTS; at a group boundary it costs a full extra slot) ----
            def body_pair(n, m, wn, last=False):
                tra = trb = None
                for mm in (m, m + 1):
                    py = py_pool.tile([P, NT], f32, name=f"py{n}_{mm}", tag="py")
                    for k in range(KT):
                        nc.tensor.matmul(
                            py[:],
                            at_tiles[k][:, mm * P:(mm + 1) * P],
                            wn[:, k, :],
                            start=(k == 0), stop=(k == KT - 1),
                        )
                        if mm == m and k == KT // 2:
                            tra, trb = tmpr_pair(n, m)
                    epilogue(n, mm, py, tra if mm == m else trb,
                             split=(last and mm == m + 1))

            # n = 1..7
            for n in range(1, NTS):
                wn = w_pool.tile([P, KT, NT], bf16, name=f"w{n}", tag="wbig")
                nc.sync.dma_start(wn[:, 0:16, :], w_d[n, :, 0:16, :])
                nc.sync.dma_start(wn[:, 16:32, :], w_d[n, :, 16:32, :])
                for m in range(0, MT, 2):
                    body_pair(n, m, wn, last=(n == NTS - 1 and m == MT - 2))

    nc.finalize()
    return nc


def _get_program():
    if "nc" not in _CACHE:
        _CACHE["nc"] = _build_program()
    return _CACHE["nc"]


def kernel(x, cluster, weight, style_L, style_R):
    import os

    # The NTFF trace path needs an antenv hook this container lacks; never
    # let a stray BASS_TRACE env take the run down that path.
    os.environ.setdefault("BASS_NEVER_TRACE", "1")
    from concourse.bass_utils import run_bass_kernel_spmd

    nc = _get_program()
    bf16 = ml_dtypes.bfloat16

    # W: [din, dout] -> [n, p, k, nt] partition-major for contiguous DMA
    w_bf = np.asarray(weight, dtype=np.float32).astype(bf16)
    w_r = np.ascontiguousarray(
        w_bf.reshape(KT, P, NTS, NT).transpose(2, 1, 0, 3)
    )
    # styles/cluster duplicated across both 64-row halves for row packing
    sL1 = np.asarray(style_L, dtype=np.float32).astype(bf16)
    sR1 = np.asarray(style_R, dtype=np.float32).astype(bf16)
    sL = np.ascontiguousarray(np.vstack([sL1, sL1]))
    sR = np.ascontiguousarray(np.vstack([sR1, sR1]))

    in_maps = []
    for c in range(NCORES):
        xs = np.asarray(x[c * MB:(c + 1) * MB], dtype=np.float32)
        xT = np.ascontiguousarray(xs.T).astype(bf16)          # [DIN, MB]
        # [din, mb] -> [granule, p, k-in-granule, mb]
        xT_r = np.ascontiguousarray(
            xT.reshape(KT // XG, XG, P, MB).transpose(0, 2, 1, 3)
        )
        clT1 = np.ascontiguousarray(
            np.asarray(cluster[c * MB:(c + 1) * MB], dtype=np.float32).T
        ).astype(bf16)
        clT = np.ascontiguousarray(np.vstack([clT1, clT1]))
        in_maps.append(
            {"xT": xT_r, "clusterT": clT, "weight": w_r, "style_L": sL, "style_R": sR}
        )

    res = run_bass_kernel_spmd(nc, in_maps, list(range(NCORES)))
    LAST["results"] = res
    LAST["in_maps"] = in_maps
    out = np.concatenate(
        [np.asarray(res.results[c]["out"], dtype=np.float32) for c in range(NCORES)],
        axis=0,
    )
    return out


# revision 4
# speedup vs baseline: 1.0779x; 1.0779x over previous
# Bass/Tile TRN2 kernel for nn_Conv1D_style: out = ((x * (cluster@style_L)) @ weight) * (cluster@style_R)
#
# Sharding: data-parallel over the batch dim. Each of the 8 cores gets a
# 1024-row slice of x/cluster and a full (replicated) weight/style_L/style_R.
#
# Per-core plan (M=1024 batch, K=4096 din, N=4096 dout), all matmuls bf16
# with fp32 PSUM accumulation:
#   aT[k] = xT[k] * (style_L[:, kslice].T @ clusterT)  -> bf16, SBUF-resident.
#   y[m,n] = sum_k aT[k][:, mslice].T @ W[k, nslice]   (32 accumulating MMs)
#   out[m,n] = y[m,n] * (clusterT[:, mslice].T @ style_R[:, nslice])
#
# The aT production is fused with the first n-block's accumulation (n=0,
# m=0..3 accumulate k-outer across 4 PSUM banks) so the PE never drains in
# the prologue. The K=64 style matmuls (tmpLT/tmpR) are row-packed two at a
# time via tile_position into the upper/lower 64 PE rows. ~40 warmup
# matmuls on a memset tile run during the initial DMA latency so the PE
# HAM clock-gate is at 8/8 when real matmuls start.
#
# DMA layout: xT and W are pre-arranged on the host partition-major so each
# DMA lands 6-16 KiB contiguous per SBUF partition (big packets). Queues:
#  - Sync HWDGE: the startup critical path first (clT, sL k-granule 0,
#    xT k=0) so it isn't starved by fair-share arbitration, then W in
#    k-granules, then outputs. W prefetch descriptors for n>=1 are placed
#    after an epilogue DMA so they don't stream during the (already
#    HBM-saturated) prologue.
#  - Activation HWDGE: the xT stream (8 KiB packets).
#  - GpSimd SWDGE: sL granules 1-3 and sR (off the critical path).
# Output is written bf16 (halves write traffic) and upcast on the host.

import numpy as np
import ml_dtypes

B, DIN, DOUT, NCL = 8192, 4096, 4096, 64
NCORES = 8
MB = B // NCORES          # batch rows per core
P = 128
NT = 512                  # n tile (dout cols per matmul)
KT = DIN // P             # 32 k tiles
MT = MB // P              # 8 m tiles
NTS = DOUT // NT          # 8 n tiles
FUSED = 4                 # m tiles of n=0 accumulated during the aT prologue
XG = 4                    # k tiles per xT DMA granule
WARMUP = 40               # PE warmup matmuls (N=128) during startup DMA

_CACHE = {}
LAST = {}                 # exposes the most recent BassKernelResults for test harnesses


def _build_program():
    import concourse.bacc as bacc
    import concourse.mybir as mybir
    import concourse.tile as tile

    bf16 = mybir.dt.bfloat16
    f32 = mybir.dt.float32

    nc = bacc.Bacc(None, target_bir_lowering=False, debug=False)

    # xT: [granule, partition, k-in-granule, batch]; W: [n, partition, k, nt]
    # cluster/styles arrive duplicated: rows 64-127 = rows 0-63 (row packing).
    xT_d = nc.declare_dram_parameter("xT", [KT // XG, P, XG, MB], bf16, isOutput=False)
    clT_d = nc.declare_dram_parameter("clusterT", [P, MB], bf16, isOutput=False)
    w_d = nc.declare_dram_parameter("weight", [NTS, P, KT, NT], bf16, isOutput=False)
    sL_d = nc.declare_dram_parameter("style_L", [P, DIN], bf16, isOutput=False)
    sR_d = nc.declare_dram_parameter("style_R", [P, DOUT], bf16, isOutput=False)
    out_d = nc.declare_dram_parameter("out", [MB, DOUT], bf16, isOutput=True)

    H = NCL  # 64: row-pack halves

    with tile.TileContext(nc) as tc:
        with (
            tc.tile_pool(name="const", bufs=1) as const_pool,
            tc.tile_pool(name="atp", bufs=1) as at_pool,
            tc.tile_pool(name="wp", bufs=2) as w_pool,
            tc.tile_pool(name="xp", bufs=3) as x_pool,
            tc.tile_pool(name="evp", bufs=3) as ev_pool,
            # PSUM budget (8 banks): py 4 x [128,512] (tmpR psum + y
            # accumulators) + pl 2 x [128,1024] (2 banks each) = 8.
            tc.tile_pool(name="pyp", bufs=4, space="PSUM") as py_pool,
            tc.tile_pool(name="plp", bufs=2, space="PSUM") as pl_pool,
        ):
            clT = const_pool.tile([P, MB], bf16, name="clT")
            sL = const_pool.tile([P, DIN], bf16, name="sL")
            sR = const_pool.tile([P, DOUT], bf16, name="sR")
            warm = const_pool.tile([P, P], bf16, name="warm")
            nc.gpsimd.memset(warm[:], 0.5)

            # Startup critical path at the head of the Sync queue: the first
            # tmpLT matmul needs clT + sL[:,0:1024]; at_0 needs xT k=0.
            SLG = DIN // 4
            nc.sync.dma_start(clT[:], clT_d[:])
            nc.sync.dma_start(sL[:, 0:SLG], sL_d[:, 0:SLG])
            # sL tail + sR ride the GpSimd SWDGE queue (not latency-critical)
            for g in range(1, 4):
                nc.gpsimd.dma_start(
                    sL[:, g * SLG:(g + 1) * SLG], sL_d[:, g * SLG:(g + 1) * SLG]
                )
            nc.gpsimd.dma_start(sR[:, 0:DOUT // 2], sR_d[:, 0:DOUT // 2])
            nc.gpsimd.dma_start(sR[:, DOUT // 2:], sR_d[:, DOUT // 2:])

            # ---- PE warmup: keep the PE busy from program start until the
            # gating DMAs land, so HAM un-throttles to 8/8 before real work.
            # Results go to a pl psum slot and are never read.
            wps = pl_pool.tile([P, MB], f32, name="wps", tag="pl")
            for i in range(WARMUP):
                nc.tensor.matmul(
                    wps[:, 0:P], warm[:], warm[:], start=True, stop=True
                )

            def tmpr_pair(n, m, psum_src="py"):
                """Row-packed pair: tmpR tiles for (m, m+1) at n, staged to SBUF.

                psum_src="pl" borrows a pl-pool tile (two banks) instead of two
                py slots — required in the fused prologue where all four py
                slots are held by the open accumulators.
                """
                if psum_src == "pl":
                    prp = pl_pool.tile([P, MB], f32, name=f"prf{n}_{m}", tag="pl")
                    pra, prb = prp[:, 0:NT], prp[:, NT:MB]
                else:
                    pra = py_pool.tile([P, NT], f32, name=f"pr{n}_{m}", tag="py")
                    prb = py_pool.tile([P, NT], f32, name=f"pr{n}_{m + 1}", tag="py")
                nc.tensor.matmul(
                    pra[:],
                    clT[:H, m * P:(m + 1) * P],
                    sR[:H, n * NT:(n + 1) * NT],
                    start=True, stop=True, tile_position=(0, 0),
                )
                nc.tensor.matmul(
                    prb[:],
                    clT[H:, (m + 1) * P:(m + 2) * P],
                    sR[H:, n * NT:(n + 1) * NT],
                    start=True, stop=True, tile_position=(H, 0),
                )
                tra = ev_pool.tile([P, NT], f32, name=f"tr{n}_{m}", tag="tr", bufs=6)
                trb = ev_pool.tile([P, NT], f32, name=f"tr{n}_{m + 1}", tag="tr", bufs=6)
                nc.any.tensor_copy(out=tra[:], in_=pra[:])
                nc.any.tensor_copy(out=trb[:], in_=prb[:])
                return tra, trb

            def epilogue(n, m, py, tr, split=False):
                ot = ev_pool.tile([P, NT], bf16, name=f"ot{n}_{m}", tag="ot")
                if split:
                    hw = NT // 2
                    for h in range(2):
                        s = slice(h * hw, (h + 1) * hw)
                        nc.vector.tensor_mul(out=ot[:, s], in0=py[:, s], in1=tr[:, s])
                        nc.sync.dma_start(
                            out_d[m * P:(m + 1) * P,
                                  n * NT + h * hw:n * NT + (h + 1) * hw],
                            ot[:, s],
                        )
                else:
                    nc.vector.tensor_mul(out=ot[:], in0=py[:], in1=tr[:])
                    nc.sync.dma_start(
                        out_d[m * P:(m + 1) * P, n * NT:(n + 1) * NT], ot[:]
                    )

            # ---- W for n=0, in k-granules: the first fused MM is gated on
            # only 512 KiB, behind the 768 KiB critical path on Sync ----
            w0 = w_pool.tile([P, KT, NT], bf16, name="w0", tag="wbig")
            for lo, hi in ((0, 4), (4, 16), (16, 32)):
                nc.sync.dma_start(w0[:, lo:hi, :], w_d[0, :, lo:hi, :])

            # ---- fused prologue: aT production + n0/m0..3 k-outer accumulation ----
            py_f = [
                py_pool.tile([P, NT], f32, name=f"py0_{m}", tag="py")
                for m in range(FUSED)
            ]
            at_tiles = []
            tr_f = []
            for g in range(KT // XG):
                xg = x_pool.tile([P, XG, MB], bf16, name=f"xg{g}", tag="xg")
                if g == 0:
                    # k=0 gated on 256 KiB at the head of Sync; k=1..3 on the
                    # Activation queue where the rest of xT streams
                    nc.sync.dma_start(xg[:, 0:1, :], xT_d[0][:, 0:1, :])
                    nc.scalar.dma_start(xg[:, 1:XG, :], xT_d[0][:, 1:XG, :])
                else:
                    nc.scalar.dma_start(xg[:], xT_d[g])
                for j in range(XG):
                    k = g * XG + j
                    # tmpLT: row-packed pair, both batch halves in one slot
                    pl = pl_pool.tile([P, MB], f32, name=f"pl{k}", tag="pl")
                    nc.tensor.matmul(
                        pl[:, 0:NT],
                        sL[:H, k * P:(k + 1) * P],
                        clT[:H, 0:NT],
                        start=True, stop=True, tile_position=(0, 0),
                    )
                    nc.tensor.matmul(
                        pl[:, NT:MB],
                        sL[H:, k * P:(k + 1) * P],
                        clT[H:, NT:MB],
                        start=True, stop=True, tile_position=(H, 0),
                    )
                    at_k = at_pool.tile([P, MB], bf16, name=f"at{k}", tag=f"at{k}")
                    nc.vector.tensor_mul(out=at_k[:], in0=xg[:, j, :], in1=pl[:])
                    at_tiles.append(at_k)
                    for m in range(FUSED):
                        nc.tensor.matmul(
                            py_f[m][:],
                            at_k[:, m * P:(m + 1) * P],
                            w0[:, k, :],
                            start=(k == 0), stop=(k == KT - 1),
                        )
                if g == 3:
                    # tmpR for the fused m tiles; sR arrives on the SWDGE
                    # queue by ~20us so inject after k=12
                    tr_f += tmpr_pair(0, 0, psum_src="pl")
                elif g == 5:
                    tr_f += tmpr_pair(0, 2, psum_src="pl")
            for m in range(FUSED):
                epilogue(0, m, py_f[m], tr_f[m])

            # ---- standard m-pair body: two 32-MM groups with the packed tmpR
            # pair injected mid-group (the deep MM pipeline hides its
            # LDWEIGHTS; at a group boundary it costs a full extra slot) ----
            def body_pair(n, m, wn, last=False):
                tra = trb = None
                for mm in (m, m + 1):
                    py = py_pool.tile([P, NT], f32, name=f"py{n}_{mm}", tag="py")
                    for k in range(KT):
                        nc.tensor.matmul(
                            py[:],
                            at_tiles[k][:, mm * P:(mm + 1) * P],
                            wn[:, k, :],
                            start=(k == 0), stop=(k == KT - 1),
                        )
                        if mm == m and k == KT // 2:
                            tra, trb = tmpr_pair(n, m)
                    epilogue(n, mm, py, tra if mm == m else trb,
                             split=(last and mm == m + 1))

            # W prefetch for n>=1: issue the descriptors behind an epilogue
            # DMA on the Sync queue so the stream starts only once the
            # prologue (which saturates HBM) has finished.
            wn_tiles = {}
            for n in range(1, NTS):
                wn_tiles[n] = w_pool.tile([P, KT, NT], bf16, name=f"w{n}", tag="wbig")

            def fetch_w(n):
                nc.sync.dma_start(wn_tiles[n][:, 0:16, :], w_d[n, :, 0:16, :])
                nc.sync.dma_start(wn_tiles[n][:, 16:32, :], w_d[n, :, 16:32, :])

            # rest of n=0; W prefetch runs exactly one n-block ahead (the
            # w_pool has 2 bufs, so fetching further ahead would block the
            # Sync queue — and the epilogue DMAs behind it — on buf reuse)
            fetch_w(1)
            body_pair(0, 4, w0)
            body_pair(0, 6, w0)
            # n = 1..7
            for n in range(1, NTS):
                if n + 1 < NTS:
                    fetch_w(n + 1)
                for m in range(0, MT, 2):
                    body_pair(n, m, wn_tiles[n], last=(n == NTS - 1 and m == MT - 2))

    nc.finalize()
    return nc


def _get_program():
    if "nc" not in _CACHE:
        _CACHE["nc"] = _build_program()
    return _CACHE["nc"]


def kernel(x, cluster, weight, style_L, style_R):
    import os

    # The NTFF trace path needs an antenv hook this container lacks; never
    # let a stray BASS_TRACE env take the run down that path.
    os.environ.setdefault("BASS_NEVER_TRACE", "1")
    from concourse.bass_utils import run_bass_kernel_spmd

    nc = _get_program()
    bf16 = ml_dtypes.bfloat16

    # W: [din, dout] -> [n, p, k, nt] partition-major for contiguous DMA
    w_bf = np.asarray(weight, dtype=np.float32).astype(bf16)
    w_r = np.ascontiguousarray(
        w_bf.reshape(KT, P, NTS, NT).transpose(2, 1, 0, 3)
    )
    # styles/cluster duplicated across both 64-row halves for row packing
    sL1 = np.asarray(style_L, dtype=np.float32).astype(bf16)
    sR1 = np.asarray(style_R, dtype=np.float32).astype(bf16)
    sL = np.ascontiguousarray(np.vstack([sL1, sL1]))
    sR = np.ascontiguousarray(np.vstack([sR1, sR1]))

    in_maps = []
    for c in range(NCORES):
        xs = np.asarray(x[c * MB:(c + 1) * MB], dtype=np.float32)
        xT = np.ascontiguousarray(xs.T).astype(bf16)          # [DIN, MB]
        # [din, mb] -> [granule, p, k-in-granule, mb]
        xT_r = np.ascontiguousarray(
            xT.reshape(KT // XG, XG, P, MB).transpose(0, 2, 1, 3)
        )
        clT1 = np.ascontiguousarray(
            np.asarray(cluster[c * MB:(c + 1) * MB], dtype=np.float32).T
        ).astype(bf16)
        clT = np.ascontiguousarray(np.vstack([clT1, clT1]))
        in_maps.append(
            {"xT": xT_r, "clusterT": clT, "weight": w_r, "style_L": sL, "style_R": sR}
        )

    res = run_bass_kernel_spmd(nc, in_maps, list(range(NCORES)))
    LAST["results"] = res
    LAST["in_maps"] = in_maps
    out = np.concatenate(
        [np.asarray(res.results[c]["out"], dtype=np.float32) for c in range(NCORES)],
        axis=0,
    )
    return out


# revision 7
# speedup vs baseline: 1.1049x; 1.0251x over previous
# Bass/Tile TRN2 kernel for nn_Conv1D_style: out = ((x * (cluster@style_L)) @ weight) * (cluster@style_R)
#
# Sharding: data-parallel over the batch dim. Each of the 8 cores gets a
# 1024-row slice of x/cluster and a full (replicated) weight/style_L/style_R.
#
# Per-core plan (M=1024 batch, K=4096 din, N=4096 dout), all matmuls bf16
# with fp32 PSUM accumulation:
#   aT[k] = xT[k] * (style_L[:, kslice].T @ clusterT)  -> bf16, SBUF-resident.
#   y[m,n] = sum_k aT[k][:, mslice].T @ W[k, nslice]   (32 accumulating MMs)
#   out[m,n] = y[m,n] * (clusterT[:, mslice].T @ style_R[:, nslice])
#
# The aT production is fused with the first n-block's accumulation (n=0,
# m=0..3 accumulate k-outer across 4 PSUM banks) so the PE never drains in
# the prologue. The K=64 style matmuls (tmpLT/tmpR) are row-packed two at a
# time via tile_position into the upper/lower 64 PE rows. ~40 warmup
# matmuls on a memset tile run during the initial DMA latency so the PE
# HAM clock-gate is at 8/8 when real matmuls start.
#
# Startup: everything the first k-step needs (clT | sL k0-7 | x k0 | W n0
# k0-1) is packed into ONE 1 MiB "boot" blob with 8 KiB/partition packets
# at the head of the Sync queue — small-packet DMAs lose fair-share
# arbitration against concurrent 8-16 KiB streams, so the critical path
# must be one big-packet transfer. The x/W needs for k1-7 follow on Sync
# in consumption order as 256-512 KiB granules; the bulk x stream (k8+)
# runs on the Activation queue, gated behind the boot blob by a tiny
# SBUF->SBUF DMA so it cannot crowd the boot out. sL tail and sR ride the
# GpSimd SWDGE queue. W prefetch for n>=1 is issued behind an epilogue DMA
# so it streams only after the (HBM-saturated) prologue. Output is written
# bf16 (halves write traffic) and upcast to fp32 on the host.

import numpy as np
import ml_dtypes

B, DIN, DOUT, NCL = 8192, 4096, 4096, 64
NCORES = 8
MB = B // NCORES          # batch rows per core
P = 128
NT = 512                  # n tile (dout cols per matmul)
KT = DIN // P             # 32 k tiles
MT = MB // P              # 8 m tiles
NTS = DOUT // NT          # 8 n tiles
FUSED = 4                 # m tiles of n=0 accumulated during the aT prologue
WARMUP = 40               # PE warmup matmuls (N=128) during startup DMA

_CACHE = {}
LAST = {}                 # exposes the most recent BassKernelResults for test harnesses


def _build_program():
    import concourse.bacc as bacc
    import concourse.mybir as mybir
    import concourse.tile as tile

    bf16 = mybir.dt.bfloat16
    f32 = mybir.dt.float32

    nc = bacc.Bacc(None, target_bir_lowering=False, debug=False)

    # boot: [clT (1024) | sL k0-7 (1024) | xT k0 (1024) | W n0 k0-1 (1024)]
    # xT: [partition, k, batch] so any k-range is per-partition contiguous.
    # W: [n, partition, k, nt]. cluster/styles arrive duplicated: rows
    # 64-127 = rows 0-63 (row packing).
    boot_d = nc.declare_dram_parameter("boot", [P, 4 * MB], bf16, isOutput=False)
    xT_d = nc.declare_dram_parameter("xT", [P, KT, MB], bf16, isOutput=False)
    w_d = nc.declare_dram_parameter("weight", [NTS, P, KT, NT], bf16, isOutput=False)
    sL_d = nc.declare_dram_parameter("style_L", [P, DIN - 8 * P], bf16, isOutput=False)
    sR_d = nc.declare_dram_parameter("style_R", [P, DOUT], bf16, isOutput=False)
    out_d = nc.declare_dram_parameter("out", [MB, DOUT], bf16, isOutput=True)

    H = NCL  # 64: row-pack halves
    CL0, SL0, X0, W0 = 0, MB, 2 * MB, 3 * MB  # boot column offsets

    with tile.TileContext(nc) as tc:
        with (
            tc.tile_pool(name="const", bufs=1) as const_pool,
            tc.tile_pool(name="atp", bufs=1) as at_pool,
            tc.tile_pool(name="wp", bufs=2) as w_pool,
            tc.tile_pool(name="xp", bufs=4) as x_pool,
            tc.tile_pool(name="evp", bufs=3) as ev_pool,
            # PSUM budget (8 banks): py 4 x [128,512] (tmpR psum + y
            # accumulators) + pl 2 x [128,1024] (2 banks each) = 8.
            tc.tile_pool(name="pyp", bufs=4, space="PSUM") as py_pool,
            tc.tile_pool(name="plp", bufs=2, space="PSUM") as pl_pool,
        ):
            boot = const_pool.tile([P, 4 * MB], bf16, name="boot")
            sLr = const_pool.tile([P, DIN - 8 * P], bf16, name="sLr")
            sR = const_pool.tile([P, DOUT], bf16, name="sR")
            warm = const_pool.tile([P, P], bf16, name="warm")
            gate = const_pool.tile([1, 16], bf16, name="gate")
            nc.gpsimd.memset(warm[:], 0.5)

            def sL_ap(rows, k):
                # style_L columns for k-tile k: boot for k<8, sLr after
                if k < 8:
                    return boot[rows, SL0 + k * P:SL0 + (k + 1) * P]
                return sLr[rows, (k - 8) * P:(k - 8 + 1) * P]

            # Startup-critical blob first on Sync (8 KiB packets).
            nc.sync.dma_start(boot[:], boot_d[:])
            # sL tail + sR on the GpSimd SWDGE queue (not latency-critical)
            for g in range(3):
                nc.gpsimd.dma_start(
                    sLr[:, g * MB:(g + 1) * MB], sL_d[:, g * MB:(g + 1) * MB]
                )
            nc.gpsimd.dma_start(sR[:, 0:DOUT // 2], sR_d[:, 0:DOUT // 2])
            nc.gpsimd.dma_start(sR[:, DOUT // 2:], sR_d[:, DOUT // 2:])

            # x granules: k1-7 on Sync in consumption order (below); the bulk
            # stream k8+ on Activation, gated behind the boot blob so it
            # cannot crowd it out of the DMA fabric.
            xb0 = x_pool.tile([P, 3, MB], bf16, name="xb0", tag="xg")
            xb1 = x_pool.tile([P, 4, MB], bf16, name="xb1", tag="xg")

            def x_ap(k):
                if k == 0:
                    return boot[:, X0:X0 + MB]
                if k < 4:
                    return xb0[:, k - 1, :]
                if k < 8:
                    return xb1[:, k - 4, :]
                g = (k - 8) // 4
                return xgt[g][:, (k - 8) % 4, :]

            def w0_ap(k):
                if k < 2:
                    return boot[:, W0 + k * NT:W0 + (k + 1) * NT]
                return w0[:, k - 2, :]

            # ---- PE warmup: keep the PE busy from program start until the
            # boot blob lands, so HAM un-throttles to 8/8 before real work.
            # Results go to a pl psum slot and are never read.
            wps = pl_pool.tile([P, MB], f32, name="wps", tag="pl")
            for i in range(WARMUP):
                nc.tensor.matmul(
                    wps[:, 0:P], warm[:], warm[:], start=True, stop=True
                )

            # Sync queue, consumption-ordered: boot (above), then per-k x/W
            # granules for k1..7, then the W bulk for n=0.
            w0 = w_pool.tile([P, KT - 2, NT], bf16, name="w0", tag="wbig")
            nc.sync.dma_start(xb0[:, 0:1, :], xT_d[:, 1:2, :])
            nc.sync.dma_start(xb0[:, 1:3, :], xT_d[:, 2:4, :])
            nc.sync.dma_start(w0[:, 0:2, :], w_d[0, :, 2:4, :])
            nc.sync.dma_start(xb1[:, 0:2, :], xT_d[:, 4:6, :])
            nc.sync.dma_start(w0[:, 2:6, :], w_d[0, :, 4:8, :])
            nc.sync.dma_start(xb1[:, 2:4, :], xT_d[:, 6:8, :])
            nc.sync.dma_start(w0[:, 6:14, :], w_d[0, :, 8:16, :])
            nc.sync.dma_start(w0[:, 14:30, :], w_d[0, :, 16:32, :])

            # Gate the Activation queue behind the boot blob, then stream x.
            nc.scalar.dma_start(gate[:], boot[0:1, 0:16])
            xgt = []
            for g in range(6):
                xg = x_pool.tile([P, 4, MB], bf16, name=f"xg{g}", tag="xg")
                nc.scalar.dma_start(xg[:], xT_d[:, 8 + 4 * g:12 + 4 * g, :])
                xgt.append(xg)

            def tmpr_pair(n, m, psum_src="py"):
                """Row-packed pair: tmpR tiles for (m, m+1) at n, staged to SBUF.

                psum_src="pl" borrows a pl-pool tile (two banks) instead of two
                py slots — required in the fused prologue where all four py
                slots are held by the open accumulators.
                """
                if psum_src == "pl":
                    prp = pl_pool.tile([P, MB], f32, name=f"prf{n}_{m}", tag="pl")
                    pra, prb = prp[:, 0:NT], prp[:, NT:MB]
                else:
                    pra = py_pool.tile([P, NT], f32, name=f"pr{n}_{m}", tag="py")
                    prb = py_pool.tile([P, NT], f32, name=f"pr{n}_{m + 1}", tag="py")
                nc.tensor.matmul(
                    pra[:],
                    boot[:H, CL0 + m * P:CL0 + (m + 1) * P],
                    sR[:H, n * NT:(n + 1) * NT],
                    start=True, stop=True, tile_position=(0, 0),
                )
                nc.tensor.matmul(
                    prb[:],
                    boot[H:, CL0 + (m + 1) * P:CL0 + (m + 2) * P],
                    sR[H:, n * NT:(n + 1) * NT],
                    start=True, stop=True, tile_position=(H, 0),
                )
                tra = ev_pool.tile([P, NT], f32, name=f"tr{n}_{m}", tag="tr", bufs=6)
                trb = ev_pool.tile([P, NT], f32, name=f"tr{n}_{m + 1}", tag="tr", bufs=6)
                nc.any.tensor_copy(out=tra[:], in_=pra[:])
                nc.any.tensor_copy(out=trb[:], in_=prb[:])
                return tra, trb

            def epilogue(n, m, py, tr, split=False):
                ot = ev_pool.tile([P, NT], bf16, name=f"ot{n}_{m}", tag="ot")
                if split:
                    hw = NT // 2
                    for h in range(2):
                        s = slice(h * hw, (h + 1) * hw)
                        nc.vector.tensor_mul(out=ot[:, s], in0=py[:, s], in1=tr[:, s])
                        nc.sync.dma_start(
                            out_d[m * P:(m + 1) * P,
                                  n * NT + h * hw:n * NT + (h + 1) * hw],
                            ot[:, s],
                        )
                else:
                    nc.vector.tensor_mul(out=ot[:], in0=py[:], in1=tr[:])
                    nc.sync.dma_start(
                        out_d[m * P:(m + 1) * P, n * NT:(n + 1) * NT], ot[:]
                    )

            # ---- fused prologue: aT production + n0/m0..3 k-outer accumulation ----
            py_f = [
                py_pool.tile([P, NT], f32, name=f"py0_{m}", tag="py")
                for m in range(FUSED)
            ]
            at_tiles = []
            tr_f = []
            for k in range(KT):
                # tmpLT: row-packed pair, both batch halves in one slot
                pl = pl_pool.tile([P, MB], f32, name=f"pl{k}", tag="pl")
                nc.tensor.matmul(
                    pl[:, 0:NT],
                    sL_ap(slice(0, H), k),
                    boot[:H, CL0:CL0 + NT],
                    start=True, stop=True, tile_position=(0, 0),
                )
                nc.tensor.matmul(
                    pl[:, NT:MB],
                    sL_ap(slice(H, P), k),
                    boot[H:, CL0 + NT:CL0 + MB],
                    start=True, stop=True, tile_position=(H, 0),
                )
                at_k = at_pool.tile([P, MB], bf16, name=f"at{k}", tag=f"at{k}")
                nc.vector.tensor_mul(out=at_k[:], in0=x_ap(k), in1=pl[:])
                at_tiles.append(at_k)
                for m in range(FUSED):
                    nc.tensor.matmul(
                        py_f[m][:],
                        at_k[:, m * P:(m + 1) * P],
                        w0_ap(k),
                        start=(k == 0), stop=(k == KT - 1),
                    )
                if k == 15:
                    # tmpR for the fused m tiles; sR arrives on the SWDGE
                    # queue by ~20us
                    tr_f += tmpr_pair(0, 0, psum_src="pl")
                elif k == 23:
                    tr_f += tmpr_pair(0, 2, psum_src="pl")
            for m in range(FUSED):
                epilogue(0, m, py_f[m], tr_f[m])

            # ---- standard m-pair body: two 32-MM groups with the packed tmpR
            # pair injected mid-group (the deep MM pipeline hides its
            # LDWEIGHTS; at a group boundary it costs a full extra slot) ----
            def body_pair(n, m, w_ap, last=False):
                tra = trb = None
                for mm in (m, m + 1):
                    py = py_pool.tile([P, NT], f32, name=f"py{n}_{mm}", tag="py")
                    for k in range(KT):
                        nc.tensor.matmul(
                            py[:],
                            at_tiles[k][:, mm * P:(mm + 1) * P],
                            w_ap(k),
                            start=(k == 0), stop=(k == KT - 1),
                        )
                        if mm == m and k == KT // 2:
                            tra, trb = tmpr_pair(n, m)
                    epilogue(n, mm, py, tra if mm == m else trb,
                             split=(last and mm == m + 1))

            # W prefetch for n>=1: issue the descriptors behind an epilogue
            # DMA on the Sync queue so the stream starts only once the
            # prologue (which saturates HBM) has finished; prefetch exactly
            # one n ahead (w_pool bufs=2).
            wn_tiles = {}
            for n in range(1, NTS):
                wn_tiles[n] = w_pool.tile([P, KT, NT], bf16, name=f"w{n}", tag="wbig")

            def fetch_w(n):
                nc.sync.dma_start(wn_tiles[n][:, 0:16, :], w_d[n, :, 0:16, :])
                nc.sync.dma_start(wn_tiles[n][:, 16:32, :], w_d[n, :, 16:32, :])

            fetch_w(1)
            body_pair(0, 4, w0_ap)
            body_pair(0, 6, w0_ap)
            # n = 1..7
            for n in range(1, NTS):
                if n + 1 < NTS:
                    fetch_w(n + 1)
                wt = wn_tiles[n]
                w_ap = lambda k, wt=wt: wt[:, k, :]
                for m in range(0, MT, 2):
                    body_pair(n, m, w_ap, last=(n == NTS - 1 and m == MT - 2))

    nc.finalize()
    return nc


def _get_program():
    if "nc" not in _CACHE:
        _CACHE["nc"] = _build_program()
    return _CACHE["nc"]


def kernel(x, cluster, weight, style_L, style_R):
    import os

    # The NTFF trace path needs an antenv hook this container lacks; never
    # let a stray BASS_TRACE env take the run down that path.
    os.environ.setdefault("BASS_NEVER_TRACE", "1")
    from concourse.bass_utils import run_bass_kernel_spmd

    nc = _get_program()
    bf16 = ml_dtypes.bfloat16

    # W: [din, dout] -> [n, p, k, nt] partition-major for contiguous DMA
    w_bf = np.asarray(weight, dtype=np.float32).astype(bf16)
    w_r = np.ascontiguousarray(
        w_bf.reshape(KT, P, NTS, NT).transpose(2, 1, 0, 3)
    )
    # styles/cluster duplicated across both 64-row halves for row packing
    sL1 = np.asarray(style_L, dtype=np.float32).astype(bf16)
    sR1 = np.asarray(style_R, dtype=np.float32).astype(bf16)
    sL = np.ascontiguousarray(np.vstack([sL1, sL1]))
    sR = np.ascontiguousarray(np.vstack([sR1, sR1]))
    sL_tail = np.ascontiguousarray(sL[:, 8 * P:])
    w_boot = np.ascontiguousarray(w_r[0][:, 0:2, :].reshape(P, 2 * NT))

    in_maps = []
    for c in range(NCORES):
        xs = np.asarray(x[c * MB:(c + 1) * MB], dtype=np.float32)
        xT = np.ascontiguousarray(xs.T).astype(bf16)          # [DIN, MB]
        xT_r = np.ascontiguousarray(
            xT.reshape(KT, P, MB).transpose(1, 0, 2)          # [P, KT, MB]
        )
        clT1 = np.ascontiguousarray(
            np.asarray(cluster[c * MB:(c + 1) * MB], dtype=np.float32).T
        ).astype(bf16)
        clT = np.ascontiguousarray(np.vstack([clT1, clT1]))
        boot = np.ascontiguousarray(
            np.concatenate([clT, sL[:, 0:8 * P], xT_r[:, 0, :], w_boot], axis=1)
        )
        in_maps.append(
            {"boot": boot, "xT": xT_r, "weight": w_r,
             "style_L": sL_tail, "style_R": sR}
        )

    res = run_bass_kernel_spmd(nc, in_maps, list(range(NCORES)))
    LAST["results"] = res
    LAST["in_maps"] = in_maps
    out = np.concatenate(
        [np.asarray(res.results[c]["out"], dtype=np.float32) for c in range(NCORES)],
        axis=0,
    )
    return out


# revision 9
# speedup vs baseline: 1.1120x; 1.0064x over previous
# Bass/Tile TRN2 kernel for nn_Conv1D_style: out = ((x * (cluster@style_L)) @ weight) * (cluster@style_R)
#
# Sharding: data-parallel over the batch dim. Each of the 8 cores gets a
# 1024-row slice of x/cluster and a full (replicated) weight/style_L/style_R.
#
# Per-core plan (M=1024 batch, K=4096 din, N=4096 dout), all matmuls bf16
# with fp32 PSUM accumulation:
#   aT[k] = xT[k] * (style_L[:, kslice].T @ clusterT)  -> bf16, SBUF-resident.
#   y[m,n] = sum_k aT[k][:, mslice].T @ W[k, nslice]   (32 accumulating MMs)
#   out[m,n] = y[m,n] * (clusterT[:, mslice].T @ style_R[:, nslice])
#
# The aT production is fused with the first n-block's accumulation (n=0,
# m=0..3 accumulate k-outer across 4 PSUM banks) so the PE never drains in
# the prologue. The K=64 style matmuls (tmpLT/tmpR) are row-packed two at a
# time via tile_position into the upper/lower 64 PE rows. ~40 warmup
# matmuls on a memset tile run during the initial DMA latency so the PE
# HAM clock-gate is at 8/8 when real matmuls start.
#
# Startup: everything the first k-step needs (clT | sL k0-7 | x k0 | W n0
# k0-1) is packed into ONE 1 MiB "boot" blob with 8 KiB/partition packets
# at the head of the Sync queue — small-packet DMAs lose fair-share
# arbitration against concurrent 8-16 KiB streams, so the critical path
# must be one big-packet transfer. The x/W needs for k1-7 follow on Sync
# in consumption order as 256-512 KiB granules; the bulk x stream (k8+)
# runs on the Activation queue, gated behind the boot blob by a tiny
# SBUF->SBUF DMA so it cannot crowd the boot out. sL tail and sR ride the
# GpSimd SWDGE queue. W prefetch for n>=1 is issued behind an epilogue DMA
# so it streams only after the (HBM-saturated) prologue. Output is written
# bf16 (halves write traffic) and upcast to fp32 on the host.

import numpy as np
import ml_dtypes

B, DIN, DOUT, NCL = 8192, 4096, 4096, 64
NCORES = 8
MB = B // NCORES          # batch rows per core
P = 128
NT = 512                  # n tile (dout cols per matmul)
KT = DIN // P             # 32 k tiles
MT = MB // P              # 8 m tiles
NTS = DOUT // NT          # 8 n tiles
FUSED = 4                 # m tiles of n=0 accumulated during the aT prologue
WARMUP = 50               # PE warmup matmuls (N=128) during startup DMA

_CACHE = {}
LAST = {}                 # exposes the most recent BassKernelResults for test harnesses


def _build_program():
    import concourse.bacc as bacc
    import concourse.mybir as mybir
    import concourse.tile as tile

    bf16 = mybir.dt.bfloat16
    f32 = mybir.dt.float32

    nc = bacc.Bacc(None, target_bir_lowering=False, debug=False)

    # boot: [clT (1024) | sL k0-7 (1024) | xT k0 (1024) | W n0 k0-1 (1024)]
    # xT: [partition, k, batch] so any k-range is per-partition contiguous.
    # W: [n, partition, k, nt]. cluster/styles arrive duplicated: rows
    # 64-127 = rows 0-63 (row packing).
    boot_d = nc.declare_dram_parameter("boot", [P, 4 * MB], bf16, isOutput=False)
    xT_d = nc.declare_dram_parameter("xT", [P, KT, MB], bf16, isOutput=False)
    w_d = nc.declare_dram_parameter("weight", [NTS, P, KT, NT], bf16, isOutput=False)
    sL_d = nc.declare_dram_parameter("style_L", [P, DIN - 8 * P], bf16, isOutput=False)
    sR_d = nc.declare_dram_parameter("style_R", [P, DOUT], bf16, isOutput=False)
    out_d = nc.declare_dram_parameter("out", [MB, DOUT], bf16, isOutput=True)

    H = NCL  # 64: row-pack halves
    CL0, SL0, X0, W0 = 0, MB, 2 * MB, 3 * MB  # boot column offsets

    with tile.TileContext(nc) as tc:
        with (
            tc.tile_pool(name="const", bufs=1) as const_pool,
            tc.tile_pool(name="atp", bufs=1) as at_pool,
            tc.tile_pool(name="wp", bufs=2) as w_pool,
            tc.tile_pool(name="xp", bufs=4) as x_pool,
            tc.tile_pool(name="evp", bufs=3) as ev_pool,
            # PSUM budget (8 banks): py 4 x [128,512] (tmpR psum + y
            # accumulators) + pl 2 x [128,1024] (2 banks each) = 8.
            tc.tile_pool(name="pyp", bufs=4, space="PSUM") as py_pool,
            tc.tile_pool(name="plp", bufs=2, space="PSUM") as pl_pool,
        ):
            boot = const_pool.tile([P, 4 * MB], bf16, name="boot")
            sLr = const_pool.tile([P, DIN - 8 * P], bf16, name="sLr")
            sR = const_pool.tile([P, DOUT], bf16, name="sR")
            warm = const_pool.tile([P, P], bf16, name="warm")
            gate = const_pool.tile([1, 16], bf16, name="gate")
            nc.gpsimd.memset(warm[:], 0.5)

            def sL_ap(rows, k):
                # style_L columns for k-tile k: boot for k<8, sLr after
                if k < 8:
                    return boot[rows, SL0 + k * P:SL0 + (k + 1) * P]
                return sLr[rows, (k - 8) * P:(k - 8 + 1) * P]

            # Startup-critical blob first on Sync (8 KiB packets).
            nc.sync.dma_start(boot[:], boot_d[:])
            # sL tail + sR on the GpSimd SWDGE queue (not latency-critical)
            for g in range(3):
                nc.gpsimd.dma_start(
                    sLr[:, g * MB:(g + 1) * MB], sL_d[:, g * MB:(g + 1) * MB]
                )
            nc.gpsimd.dma_start(sR[:, 0:DOUT // 2], sR_d[:, 0:DOUT // 2])
            nc.gpsimd.dma_start(sR[:, DOUT // 2:], sR_d[:, DOUT // 2:])

            # x granules: k1-7 on Sync in consumption order (below); the bulk
            # stream k8+ on Activation, gated behind the boot blob so it
            # cannot crowd it out of the DMA fabric.
            xb0 = x_pool.tile([P, 3, MB], bf16, name="xb0", tag="xg")
            xb1 = x_pool.tile([P, 4, MB], bf16, name="xb1", tag="xg")

            def x_ap(k):
                if k == 0:
                    return boot[:, X0:X0 + MB]
                if k < 4:
                    return xb0[:, k - 1, :]
                if k < 8:
                    return xb1[:, k - 4, :]
                g = (k - 8) // 4
                return xgt[g][:, (k - 8) % 4, :]

            def w0_ap(k):
                if k < 2:
                    return boot[:, W0 + k * NT:W0 + (k + 1) * NT]
                return w0[:, k - 2, :]

            # ---- PE warmup: keep the PE busy from program start until the
            # boot blob lands, so HAM un-throttles to 8/8 before real work.
            # Results go to a pl psum slot and are never read.
            wps = pl_pool.tile([P, MB], f32, name="wps", tag="pl")
            for i in range(WARMUP):
                nc.tensor.matmul(
                    wps[:, 0:P], warm[:], warm[:], start=True, stop=True
                )

            # Sync queue, consumption-ordered: boot (above), then per-k x/W
            # granules for k1..7, then the W bulk for n=0.
            w0 = w_pool.tile([P, KT - 2, NT], bf16, name="w0", tag="wbig")
            nc.sync.dma_start(xb0[:, 0:1, :], xT_d[:, 1:2, :])
            nc.sync.dma_start(w0[:, 0:2, :], w_d[0, :, 2:4, :])
            nc.sync.dma_start(xb0[:, 1:3, :], xT_d[:, 2:4, :])
            nc.sync.dma_start(w0[:, 2:6, :], w_d[0, :, 4:8, :])
            nc.sync.dma_start(xb1[:, 0:2, :], xT_d[:, 4:6, :])
            nc.sync.dma_start(w0[:, 6:8, :], w_d[0, :, 8:10, :])
            nc.sync.dma_start(xb1[:, 2:4, :], xT_d[:, 6:8, :])
            nc.sync.dma_start(w0[:, 8:14, :], w_d[0, :, 10:16, :])
            nc.sync.dma_start(w0[:, 14:30, :], w_d[0, :, 16:32, :])

            # Gate the Activation queue behind the last Sync x-granule (k6-7)
            # so the bulk x stream cannot crowd the per-k startup granules
            # out of the DMA fabric; it has until ~k16 to catch up.
            nc.scalar.dma_start(gate[:], xb1[0:1, 3, 0:16])
            xgt = []
            for g in range(6):
                xg = x_pool.tile([P, 4, MB], bf16, name=f"xg{g}", tag="xg")
                nc.scalar.dma_start(xg[:], xT_d[:, 8 + 4 * g:12 + 4 * g, :])
                xgt.append(xg)

            def tmpr_pair(n, m, psum_src="py"):
                """Row-packed pair: tmpR tiles for (m, m+1) at n, staged to SBUF.

                psum_src="pl" borrows a pl-pool tile (two banks) instead of two
                py slots — required in the fused prologue where all four py
                slots are held by the open accumulators.
                """
                if psum_src == "pl":
                    prp = pl_pool.tile([P, MB], f32, name=f"prf{n}_{m}", tag="pl")
                    pra, prb = prp[:, 0:NT], prp[:, NT:MB]
                else:
                    pra = py_pool.tile([P, NT], f32, name=f"pr{n}_{m}", tag="py")
                    prb = py_pool.tile([P, NT], f32, name=f"pr{n}_{m + 1}", tag="py")
                nc.tensor.matmul(
                    pra[:],
                    boot[:H, CL0 + m * P:CL0 + (m + 1) * P],
                    sR[:H, n * NT:(n + 1) * NT],
                    start=True, stop=True, tile_position=(0, 0),
                )
                nc.tensor.matmul(
                    prb[:],
                    boot[H:, CL0 + (m + 1) * P:CL0 + (m + 2) * P],
                    sR[H:, n * NT:(n + 1) * NT],
                    start=True, stop=True, tile_position=(H, 0),
                )
                tra = ev_pool.tile([P, NT], f32, name=f"tr{n}_{m}", tag="tr", bufs=6)
                trb = ev_pool.tile([P, NT], f32, name=f"tr{n}_{m + 1}", tag="tr", bufs=6)
                nc.any.tensor_copy(out=tra[:], in_=pra[:])
                nc.any.tensor_copy(out=trb[:], in_=prb[:])
                return tra, trb

            def epilogue(n, m, py, tr, split=False):
                ot = ev_pool.tile([P, NT], bf16, name=f"ot{n}_{m}", tag="ot")
                if split:
                    hw = NT // 2
                    for h in range(2):
                        s = slice(h * hw, (h + 1) * hw)
                        nc.vector.tensor_mul(out=ot[:, s], in0=py[:, s], in1=tr[:, s])
                        nc.sync.dma_start(
                            out_d[m * P:(m + 1) * P,
                                  n * NT + h * hw:n * NT + (h + 1) * hw],
                            ot[:, s],
                        )
                else:
                    nc.vector.tensor_mul(out=ot[:], in0=py[:], in1=tr[:])
                    nc.sync.dma_start(
                        out_d[m * P:(m + 1) * P, n * NT:(n + 1) * NT], ot[:]
                    )

            # ---- fused prologue: aT production + n0/m0..3 k-outer accumulation ----
            py_f = [
                py_pool.tile([P, NT], f32, name=f"py0_{m}", tag="py")
                for m in range(FUSED)
            ]
            at_tiles = []
            tr_f = []
            for k in range(KT):
                # tmpLT: row-packed pair, both batch halves in one slot
                pl = pl_pool.tile([P, MB], f32, name=f"pl{k}", tag="pl")
                nc.tensor.matmul(
                    pl[:, 0:NT],
                    sL_ap(slice(0, H), k),
                    boot[:H, CL0:CL0 + NT],
                    start=True, stop=True, tile_position=(0, 0),
                )
                nc.tensor.matmul(
                    pl[:, NT:MB],
                    sL_ap(slice(H, P), k),
                    boot[H:, CL0 + NT:CL0 + MB],
                    start=True, stop=True, tile_position=(H, 0),
                )
                at_k = at_pool.tile([P, MB], bf16, name=f"at{k}", tag=f"at{k}")
                nc.vector.tensor_mul(out=at_k[:], in0=x_ap(k), in1=pl[:])
                at_tiles.append(at_k)
                for m in range(FUSED):
                    nc.tensor.matmul(
                        py_f[m][:],
                        at_k[:, m * P:(m + 1) * P],
                        w0_ap(k),
                        start=(k == 0), stop=(k == KT - 1),
                    )
                if k == 15:
                    # tmpR for the fused m tiles; sR arrives on the SWDGE
                    # queue by ~20us
                    tr_f += tmpr_pair(0, 0, psum_src="pl")
                elif k == 23:
                    tr_f += tmpr_pair(0, 2, psum_src="pl")
            for m in range(FUSED):
                epilogue(0, m, py_f[m], tr_f[m])

            # ---- standard m-pair body: two 32-MM groups with the packed tmpR
            # pair injected mid-group (the deep MM pipeline hides its
            # LDWEIGHTS; at a group boundary it costs a full extra slot) ----
            def body_pair(n, m, w_ap, last=False):
                tra = trb = None
                for mm in (m, m + 1):
                    py = py_pool.tile([P, NT], f32, name=f"py{n}_{mm}", tag="py")
                    for k in range(KT):
                        nc.tensor.matmul(
                            py[:],
                            at_tiles[k][:, mm * P:(mm + 1) * P],
                            w_ap(k),
                            start=(k == 0), stop=(k == KT - 1),
                        )
                        if mm == m and k == KT // 2:
                            tra, trb = tmpr_pair(n, m)
                    epilogue(n, mm, py, tra if mm == m else trb,
                             split=(last and mm == m + 1))

            # W prefetch for n>=1: issue the descriptors behind an epilogue
            # DMA on the Sync queue so the stream starts only once the
            # prologue (which saturates HBM) has finished; prefetch exactly
            # one n ahead (w_pool bufs=2).
            wn_tiles = {}
            for n in range(1, NTS):
                wn_tiles[n] = w_pool.tile([P, KT, NT], bf16, name=f"w{n}", tag="wbig")

            def fetch_w(n):
                nc.sync.dma_start(wn_tiles[n][:, 0:16, :], w_d[n, :, 0:16, :])
                nc.sync.dma_start(wn_tiles[n][:, 16:32, :], w_d[n, :, 16:32, :])

            fetch_w(1)
            body_pair(0, 4, w0_ap)
            body_pair(0, 6, w0_ap)
            # n = 1..7
            for n in range(1, NTS):
                if n + 1 < NTS:
                    fetch_w(n + 1)
                wt = wn_tiles[n]
                w_ap = lambda k, wt=wt: wt[:, k, :]
                for m in range(0, MT, 2):
                    body_pair(n, m, w_ap, last=(n == NTS - 1 and m == MT - 2))

    nc.finalize()
    return nc


def _get_program():
    if "nc" not in _CACHE:
        _CACHE["nc"] = _build_program()
    return _CACHE["nc"]


def kernel(x, cluster, weight, style_L, style_R):
    import os

    # The NTFF trace path needs an antenv hook this container lacks; never
    # let a stray BASS_TRACE env take the run down that path.
    os.environ.setdefault("BASS_NEVER_TRACE", "1")
    from concourse.bass_utils import run_bass_kernel_spmd

    nc = _get_program()
    bf16 = ml_dtypes.bfloat16

    # W: [din, dout] -> [n, p, k, nt] partition-major for contiguous DMA
    w_bf = np.asarray(weight, dtype=np.float32).astype(bf16)
    w_r = np.ascontiguousarray(
        w_bf.reshape(KT, P, NTS, NT).transpose(2, 1, 0, 3)
    )
    # styles/cluster duplicated across both 64-row halves for row packing
    sL1 = np.asarray(style_L, dtype=np.float32).astype(bf16)
    sR1 = np.asarray(style_R, dtype=np.float32).astype(bf16)
    sL = np.ascontiguousarray(np.vstack([sL1, sL1]))
    sR = np.ascontiguousarray(np.vstack([sR1, sR1]))
    sL_tail = np.ascontiguousarray(sL[:, 8 * P:])
    w_boot = np.ascontiguousarray(w_r[0][:, 0:2, :].reshape(P, 2 * NT))

    in_maps = []
    for c in range(NCORES):
        xs = np.asarray(x[c * MB:(c + 1) * MB], dtype=np.float32)
        xT = np.ascontiguousarray(xs.T).astype(bf16)          # [DIN, MB]
        xT_r = np.ascontiguousarray(
            xT.reshape(KT, P, MB).transpose(1, 0, 2)          # [P, KT, MB]
        )
        clT1 = np.ascontiguousarray(
            np.asarray(cluster[c * MB:(c + 1) * MB], dtype=np.float32).T
        ).astype(bf16)
        clT = np.ascontiguousarray(np.vstack([clT1, clT1]))
        boot = np.ascontiguousarray(
            np.concatenate([clT, sL[:, 0:8 * P], xT_r[:, 0, :], w_boot], axis=1)
        )
        in_maps.append(
            {"boot": boot, "xT": xT_r, "weight": w_r,
             "style_L": sL_tail, "style_R": sR}
        )

    res = run_bass_kernel_spmd(nc, in_maps, list(range(NCORES)))
    LAST["results"] = res
    LAST["in_maps"] = in_maps
    out = np.concatenate(
        [np.asarray(res.results[c]["out"], dtype=np.float32) for c in range(NCORES)],
        axis=0,
    )
    return out
